# revision 74
# baseline (speedup 1.0000x reference)
"""Policy-masked sparse attention on 8 trn2 NeuronCores — fp8 DoubleRow.

Data-parallel over B (one batch element per core).  The 768-deep matmuls
(QKV, V, T-accumulate, proj) run as fp8e4m3 DoubleRow.  The 64-deep S
matmul stays fp16 and is row-tiled: heads 2f/2f+1 occupy partitions
0:64 / 64:128 of the K/Q chunk tiles, so their score matmuls issue to row
groups (0,0)/(64,0) and run concurrently in the PE array.

Softmax post-processing is the machine bottleneck (every S element must
pass through Scalar or Vector once — the only engines with a PSUM port):
  - exp blocks split per EXP_TABLE: Scalar true Exp (fp8 out) vs Vector
    Schraudolph (E = bits((log2e*S + (7-c))*8) as int8 ≡ fp8e4m3), paired
    so the two heads of a score block mostly run on different engines.
  - V_aug carries pol in col 0, V in cols 64:128, so the denominator row
    lands on T-psum partition 0 where reciprocal_approx_fast can read it
    directly (partition-0-only op); GpSimd broadcasts it; Vector scales.
  - 4 rotating 2-bank PSUM buffers shared by score pairs, T-accumulates
    and all projection work; per-pair schedule interleaves QKV/V/drop/T
    fillers (each <=1us of PE) between score blocks.
  - ~4.8us of dummy matmuls during the input-DMA wait open the HAM clock
    gate before real work; a duplicated contiguous Q0K0 weight block plus
    per-pb x transfers shorten the critical DMA path.

Scales (power-of-2, folded on host):
  Wq' = Wq*hd^-0.5*64, Wk' = Wk*64, Wv' = Wv*64, x fp8 plain
  QKV psum = 64*q ; Q/K cast to fp16 -> 64*q ; S psum = 4096*S
  exp scale 1/4096 ; V stored fp8 64*v ; T psum = 64*num ; D = pol row
  OAT = T/D = 64*attn (fp8) ; Wp' = 64*Wproj ; proj psum = 4096*out
  final scalar_tensor_tensor: out = psum/4096 + bias
Drop path (out = x@W2.T + b) stays fp16 (fp8 measured 3.2% rel err).
Measured: 102.9us HW exec, rel err 2.8e-3 (baseline 145.5us).
"""

import math
import os
import numpy as np
import ml_dtypes

import concourse.bass as bass
import concourse.bacc as bacc
import concourse.mybir as mybir
from concourse import tile
from concourse.bass_utils import run_bass_kernel_spmd

C = 768
H = 12
HD = 64
CB = C // 128          # 6 feature blocks of 128
PB = C // 256          # 3 feature pair-blocks of 256
VW = 128               # V_aug per-head width: pol col 0, V cols 64:128
F8 = mybir.dt.float8e4
F16 = mybir.dt.float16
F32 = mybir.dt.float32
I8 = mybir.dt.int8
DR = mybir.MatmulPerfMode.DoubleRow
f8np = ml_dtypes.float8_e4m3

SCH_C = 0.05
SCH_A = math.log2(math.e) * 8.0
SCH_B = (7.0 - SCH_C) * 8.0
SDEN = 4096.0          # S psum carries 4096*S (Q,K held fp16 at 64x)

_cache = {}


def _groups(n, limit=512):
    out = []
    off = 0
    while off < n:
        g = min(limit, n - off)
        out.append((off, g))
        off += g
    return out


# exp engine split per (pair, kb): 2 chars for (even head, odd head),
# 'S' = scalar true-exp, 'V' = vector Schraudolph.
EXP_TABLE = {
    0: ["VV", "SV", "VS", "SV", "VS", "SV"],
    1: ["VS", "SV", "VS", "SV", "VS", "SV"],
    2: ["VS", "SV", "SS", "VS", "SS", "SV"],
    3: ["VS", "SV", "SS", "VS", "SS", "SV"],
    4: ["VS", "SV", "SS", "VS", "SS", "SV"],
    5: ["SS", "SV", "SS", "VS", "SS", "SV"],
}
# v_chunk cast engine per tb
VCAST_ENG = "VVSSSS"
# qkv dest-copy engine per chunk j (0..5 Q, 6..11 K): 'S'/'V'
QKV_COPY_ENG = "SSSSSS VSSSSS".replace(" ", "")


def _build(NK, ND, NKM):
    KB = NK // 128
    KBP = KB // 2
    DB = ND // 128
    nc = bacc.Bacc("TRN2", target_bir_lowering=False, debug=False,
                   num_devices=8)

    x8p = nc.dram_tensor("x8p", [128, PB, 2, NK], F8, kind="ExternalInput").ap()
    # wqkv8 col order: [Q0,K0,Q1,K1,...,Q5,K5, V]  (256-col QK pairs first)
    wqkv8 = nc.dram_tensor("wqkv8", [128, PB, 2, 3 * C], F8,
                           kind="ExternalInput").ap()
    # first-wave copy of the Q0K0 weight block, contiguous for one fast DMA
    wqa = nc.dram_tensor("wqa", [128, PB, 2, 256], F8,
                         kind="ExternalInput").ap()
    wp8 = nc.dram_tensor("wp8", [128, PB, 2, C], F8, kind="ExternalInput").ap()
    xdT = nc.dram_tensor("xdT", [128, CB, ND], F16, kind="ExternalInput").ap()
    w2T = nc.dram_tensor("w2T", [128, CB, C], F16, kind="ExternalInput").ap()
    pol8 = nc.dram_tensor("pol8", [128, NK // 128, H], F8,
                          kind="ExternalInput").ap()
    biasb = nc.dram_tensor("biasb", [128, C], F32, kind="ExternalInput").ap()
    outk = nc.dram_tensor("outk", [NK, C], F16, kind="ExternalOutput").ap()
    outd = nc.dram_tensor("outd", [ND, C], F16, kind="ExternalOutput").ap()

    GK = _groups(NK)
    GKM = _groups(NKM)
    GC = _groups(C)
    MUL = mybir.AluOpType.mult
    ADD = mybir.AluOpType.add

    with tile.TileContext(nc) as tc:
        with (
            tc.tile_pool(name="const", bufs=1) as cpool,
            tc.tile_pool(name="ins", bufs=1) as ipool,
            tc.tile_pool(name="acts", bufs=1) as apool,
            tc.tile_pool(name="work", bufs=4) as wpool,
            tc.tile_pool(name="outs", bufs=3) as opool,
            tc.tile_pool(name="ps", bufs=4, space="PSUM") as pspool,
        ):
            # ---- scalar act-table preload (hide the 2.7us exp set load) --
            warm = cpool.tile([1, 8], F32, name="warm", tag="warm")
            nc.gpsimd.memset(warm[:], 0.0)
            warm2 = cpool.tile([1, 8], F32, name="warm2", tag="warm2")
            nc.scalar.activation(warm2[:], warm[:],
                                 mybir.ActivationFunctionType.Exp)

            # ---- inputs (DMA emission order = priority order) ----
            wqa_t = ipool.tile([128, PB, 2, 256], F8, name="wqa", tag="wqa")
            wq_t = [ipool.tile([128, 2, 3 * C], F8, name=f"wq{pb}",
                               tag=f"wq{pb}") for pb in range(PB)]
            xc_all = ipool.tile([128, PB, 2, NK], F8, name="xc", tag="xc")
            for pb in range(PB):
                nc.sync.dma_start(wqa_t[:, pb:pb + 1], wqa[:, pb:pb + 1])
                nc.scalar.dma_start(xc_all[:, pb:pb + 1], x8p[:, pb:pb + 1])

            # PE warm-up: >=3.4us of sustained dummy matmuls during the
            # DMA wait so HAM opens the clock gate before real work
            cw = cpool.tile([128, 128], F8, name="cw", tag="cw")
            nc.gpsimd.memset(cw[:], 0.0)
            wps = pspool.tile([128, 128], F32, name="wps", tag="s")
            for _ in range(45):
                nc.tensor.matmul(wps[0:16, :], lhsT=cw[:, 0:16],
                                 rhs=cw[:], start=True, stop=True)
            for pb in range(PB):
                nc.sync.dma_start(wq_t[pb][:, :, 256:2 * C],
                                  wqkv8[:, pb, :, 256:2 * C])
            pol_all = ipool.tile([128, KB, H], F8, name="pol", tag="pol")
            nc.sync.dma_start(pol_all[:], pol8)
            for pb in range(PB):
                nc.sync.dma_start(wq_t[pb][:, :, 2 * C:3 * C],
                                  wqkv8[:, pb, :, 2 * C:3 * C])
            xd_all = ipool.tile([128, CB, ND], F16, name="xd", tag="xd")
            nc.sync.dma_start(xd_all[:], xdT)
            w2_all = ipool.tile([128, CB, C], F16, name="w2", tag="w2")
            nc.sync.dma_start(w2_all[:], w2T)
            bias_t = cpool.tile([128, C], F32, name="bias", tag="bias")
            nc.sync.dma_start(bias_t[:], biasb[:])
            wp_all = ipool.tile([128, PB, 2, C], F8, name="wp", tag="wp")
            nc.sync.dma_start(wp_all[:], wp8)

            def xc_t(pb):
                return xc_all[:, pb:pb + 1].squeeze()

            def wp_t(pb):
                return wp_all[:, pb:pb + 1].squeeze()

            # ---- persistent intermediates ----
            QcT = [apool.tile([128, NKM], F16, name=f"q{j}", tag=f"q{j}")
                   for j in range(CB)]
            KcT = [apool.tile([128, NK], F16, name=f"k{j}", tag=f"k{j}")
                   for j in range(CB)]
            Vag = apool.tile([128, KBP * 2 * H * VW], F8, name="vag",
                             tag="vag")
            nc.gpsimd.memset(Vag[:], 0.0)
            OATP = [apool.tile([128, 2 * NK], F8, name=f"oat{fb}",
                               tag=f"oat{fb}") for fb in range(PB)]
            ET = {}
            for hm in range(4):
                for kbp in range(KBP):
                    ET[(hm, kbp)] = apool.tile(
                        [128, 2 * NKM], I8, name=f"et{hm}_{kbp}",
                        tag=f"et{hm}_{kbp}")

            def et3(h, kbp):
                return ET[(h % 4, kbp)][:].rearrange("p (i n) -> p i n", i=2)

            def vag5():
                return Vag[:].rearrange("p (b i h s) -> p b i h s",
                                        b=KBP, i=2, h=H)

            def oat3(fb):
                return OATP[fb][:].rearrange("p (i n) -> p i n", i=2)

            def qkv_chunk(j):
                """Output chunk j in 0..11: j<6 -> Q (NKM cols), else K."""
                isq = j < CB
                grps = GKM if isq else GK
                w = NKM if isq else NK
                jj = j if isq else j - CB
                col0 = 256 * jj + (0 if isq else 128)
                ps = pspool.tile([128, NK], F32, name="qps", tag="s")
                for pb in range(PB):
                    if jj == 0:
                        lhsT = wqa_t[:, pb, :, (0 if isq else 128):
                                     (128 if isq else 256)]
                    else:
                        lhsT = wq_t[pb][:, :, col0:col0 + 128]
                    for (o, n) in grps:
                        nc.tensor.matmul(
                            ps[:, o:o + n],
                            lhsT=lhsT,
                            rhs=xc_t(pb)[:, :, o:o + n],
                            start=(pb == 0), stop=(pb == PB - 1),
                            perf_mode=DR)
                dest = QcT[jj] if isq else KcT[jj]
                if QKV_COPY_ENG[j] == "S":
                    nc.scalar.copy(dest[:, 0:w], ps[:, 0:w])
                else:
                    nc.vector.tensor_copy(dest[:, 0:w], ps[:, 0:w])

            def v_chunk(tb):
                """token-major V chunk for kept token block tb."""
                ps = pspool.tile([128, C], F32, name="vps", tag="s")
                for pb in range(PB):
                    for (o, n) in GC:
                        nc.tensor.matmul(
                            ps[:, o:o + n],
                            lhsT=xc_t(pb)[:, :, tb * 128:(tb + 1) * 128],
                            rhs=wq_t[pb][:, :, 2 * C + o:2 * C + o + n],
                            start=(pb == 0), stop=(pb == PB - 1),
                            perf_mode=DR)
                kbp, sl = tb // 2, tb % 2
                va = vag5()[:, kbp:kbp + 1, sl:sl + 1, :, :].squeeze()
                ps3 = ps[:].rearrange("p (h s) -> p h s", s=HD)
                # pol in col 0 (so D lands on psum partition 0), V in 64:128
                if VCAST_ENG[tb] == "S":
                    nc.scalar.copy(va[:, :, 64:64 + HD], ps3)
                else:
                    nc.vector.tensor_copy(va[:, :, 64:64 + HD], ps3)
                pol3 = pol_all[:, tb:tb + 1, :].squeeze().unsqueeze(-1)
                nc.gpsimd.tensor_copy(va[:, :, 0:1], pol3)

            def s_pair(p, kb):
                """Concurrent row-tiled score matmuls for heads 2p/2p+1,
                key block kb, followed by their exps (one per engine when
                the table splits them)."""
                fc = p
                pss = []
                for half in range(2):
                    rows = slice(half * 64, half * 64 + 64)
                    ps = pspool.tile([128, NKM], F32, name="sps", tag="s")
                    pss.append(ps)
                    for (o, n) in GKM:
                        nc.tensor.matmul(
                            ps[:, o:o + n],
                            lhsT=KcT[fc][rows, kb * 128:(kb + 1) * 128],
                            rhs=QcT[fc][rows, o:o + n],
                            start=True, stop=True,
                            tile_position=(half * 64, 0))
                for half in range(2):
                    h = 2 * p + half
                    ps = pss[half]
                    dst = et3(h, kb // 2)[:, kb % 2, 0:NKM].squeeze()
                    if EXP_TABLE[p][kb][half] == "S":
                        nc.scalar.activation(
                            dst.bitcast(F8), ps[:, 0:NKM],
                            mybir.ActivationFunctionType.Exp,
                            scale=1.0 / SDEN)
                    else:
                        nc.vector.tensor_scalar(dst, ps[:, 0:NKM],
                                                SCH_A / SDEN, SCH_B,
                                                op0=MUL, op1=ADD)

            def t_acc(h, pool=None):
                ptT = pspool.tile([VW, NKM], F32, name="ptT", tag="s")
                for kbp in range(KBP):
                    lhs = vag5()[:, kbp:kbp + 1, :, h:h + 1, :].squeeze()
                    e3 = et3(h, kbp)
                    for (o, n) in GKM:
                        nc.tensor.matmul(
                            ptT[:, o:o + n],
                            lhsT=lhs,
                            rhs=e3[:, :, o:o + n].bitcast(F8),
                            start=(kbp == 0), stop=(kbp == KBP - 1),
                            perf_mode=DR)
                return ptT

            def t_fin(h, ptT):
                # tail heads: column-split halves pipeline the
                # recip -> broadcast -> scale chain (latency, not throughput)
                fb, sl, r0 = h // 4, (h // 2) % 2, (h % 2) * 64
                dst = oat3(fb)[r0:r0 + 64, sl:sl + 1, 0:NKM].squeeze()
                r_sb = wpool.tile([1, NKM], F32, name="r_sb", tag="rsb")
                rb = wpool.tile([64, NKM], F32, name="rb", tag="rb")
                chunks = (0, NKM // 2) if h >= 10 else (0,)
                cw_ = NKM // len(chunks)
                for o in chunks:
                    nc.vector.reciprocal_approx_fast(
                        r_sb[:, o:o + cw_], ptT[0:1, o:o + cw_])
                    nc.gpsimd.partition_broadcast(
                        rb[:, o:o + cw_], r_sb[:, o:o + cw_], channels=64)
                    nc.vector.tensor_tensor(
                        dst[:, o:o + cw_], ptT[64:128, o:o + cw_],
                        rb[:, o:o + cw_], op=MUL)

            def t_head(h):
                t_fin(h, t_acc(h))

            def proj_start(tb, store):
                ps = pspool.tile([128, C], F32, name="pps", tag="s")
                store[tb] = ps
                for fb in range(PB - 1):
                    o3 = oat3(fb)
                    for (o, n) in GC:
                        nc.tensor.matmul(
                            ps[:, o:o + n],
                            lhsT=o3[:, :, tb * 128:(tb + 1) * 128],
                            rhs=wp_t(fb)[:, :, o:o + n],
                            start=(fb == 0), stop=False,
                            perf_mode=DR)

            def proj_fb2_sl0(tb, store):
                # fb2 half from heads 8/9 — ready once t_fin(9) has run,
                # so it can leave the t_fin(10/11)-gated critical path
                ps = store[tb]
                o3 = oat3(PB - 1)
                for (o, n) in GC:
                    nc.tensor.matmul(
                        ps[:, o:o + n],
                        lhsT=o3[:, 0, tb * 128:(tb + 1) * 128],
                        rhs=wp_t(PB - 1)[:, 0, o:o + n],
                        start=False, stop=False)

            def proj_finish(tb, store):
                ps = store.pop(tb)
                o3 = oat3(PB - 1)
                for (o, n) in GC:
                    nc.tensor.matmul(
                        ps[:, o:o + n],
                        lhsT=o3[:, 1, tb * 128:(tb + 1) * 128],
                        rhs=wp_t(PB - 1)[:, 1, o:o + n],
                        start=False, stop=True)
                ok = opool.tile([128, C], F16, name="ok", tag="ok")
                nc.vector.scalar_tensor_tensor(ok[:], ps[:], 1.0 / 4096.0,
                                               bias_t[:], op0=MUL, op1=ADD)
                nc.sync.dma_start(outk[tb * 128:(tb + 1) * 128, :], ok[:])

            def proj_kept(tb):
                store = {}
                proj_start(tb, store)
                proj_fb2_sl0(tb, store)
                proj_finish(tb, store)

            dstore = {}

            def proj_drop_part(td, part):
                """256-col slice `part` of drop-token block td (full cb sum,
                psum allocated and released within this call)."""
                o0 = part * 256
                ps = pspool.tile([128, 256], F32, name="dps", tag="s")
                for cb in range(CB):
                    nc.tensor.matmul(
                        ps[:],
                        lhsT=xd_all[:, cb, td * 128:(td + 1) * 128],
                        rhs=w2_all[:, cb, o0:o0 + 256],
                        start=(cb == 0), stop=(cb == CB - 1))
                if part == 0:
                    dstore[td] = opool.tile([128, C], F16, name="ok",
                                            tag="ok")
                ok = dstore[td]
                nc.vector.tensor_tensor(ok[:, o0:o0 + 256], ps[:],
                                        bias_t[:, o0:o0 + 256], op=ADD)
                if part == 2:
                    ok = dstore.pop(td)
                    nc.sync.dma_start(outd[td * 128:(td + 1) * 128, :], ok[:])

            # ---- schedule ----
            # fillers[p][kb] = list of callables issued after s_pair(p, kb)
            def dp(td, part):
                if td < DB:
                    return [lambda: proj_drop_part(td, part)]
                return []
            pstore = {}
            fillers = {
                0: [[lambda: qkv_chunk(CB + 1)], [lambda: qkv_chunk(1)],
                    [lambda: v_chunk(0)], [lambda: v_chunk(1)],
                    [lambda: v_chunk(2)], [lambda: v_chunk(3)]],
                1: [[lambda: v_chunk(4)], [lambda: v_chunk(5)],
                    [lambda: qkv_chunk(CB + 2)], [lambda: qkv_chunk(2)],
                    [lambda: t_head(0)], [lambda: t_head(1)]],
                2: [[lambda: qkv_chunk(CB + 3)], [lambda: qkv_chunk(3)],
                    dp(0, 0), [lambda: t_head(2)],
                    dp(0, 1), [lambda: t_head(3)]],
                3: [dp(0, 2), [lambda: qkv_chunk(CB + 4)],
                    [lambda: qkv_chunk(4)], [lambda: t_head(4)],
                    dp(1, 0), [lambda: t_head(5)]],
                4: [dp(1, 1), [lambda: qkv_chunk(CB + 5)],
                    dp(1, 2), [lambda: t_head(6)],
                    [lambda: qkv_chunk(5)], [lambda: t_head(7)]],
                5: [dp(2, 0), dp(2, 1),
                    [lambda: t_head(8)], dp(2, 2),
                    [lambda: t_head(9)], []],
            }
            qkv_chunk(CB + 0)      # K heads 0,1
            qkv_chunk(0)           # Q heads 0,1
            for p in range(6):
                fl = fillers[p]
                for kb in range(KB):
                    s_pair(p, kb)
                    if kb < len(fl):
                        for f in fl[kb]:
                            f()
            # tail: both T-accumulates back-to-back on PE, then the proj
            # fb0/fb1 partials run under the t_fin vector chains
            pt10 = t_acc(10)
            pt11 = t_acc(11)
            proj_start(0, pstore)
            proj_start(1, pstore)
            proj_start(2, pstore)
            proj_fb2_sl0(0, pstore)
            proj_fb2_sl0(1, pstore)
            proj_fb2_sl0(2, pstore)
            t_fin(10, pt10)
            t_fin(11, pt11)
            proj_finish(0, pstore)
            proj_finish(1, pstore)
            proj_finish(2, pstore)
            for tb in range(3, KB):
                proj_kept(tb)

    nc.compile()
    return nc


def kernel(x, policy, Wqkv, Wproj, bproj, _trace=False, _tmpdir=None):
    x = np.asarray(x)
    policy = np.asarray(policy)
    Wqkv = np.asarray(Wqkv, dtype=np.float32)
    Wproj = np.asarray(Wproj, dtype=np.float32)
    bproj = np.asarray(bproj, dtype=np.float32)
    B, N, _ = x.shape
    assert B == 8 and x.shape[2] == C

    pol = policy[:, :, 0] > 0.5
    kept = [np.nonzero(pol[b])[0] for b in range(B)]
    drop = [np.nonzero(~pol[b])[0] for b in range(B)]
    nk = [len(i) for i in kept]
    nd = [len(i) for i in drop]
    NK = max(256, int(math.ceil(max(nk) / 256.0)) * 256)
    ND = max(128, int(math.ceil(max(nd) / 128.0)) * 128)
    NKM = min(NK, max(128, int(math.ceil(max(nk) / 32.0)) * 32))

    key = (NK, ND, NKM)
    if key not in _cache:
        _cache[key] = _build(NK, ND, NKM)
    nc = _cache[key]

    # ---- shared weight prep ----
    Wq_s = Wqkv[:C] * (HD ** -0.5) * 64.0
    Wk_s = Wqkv[C:2 * C] * 64.0
    Wv_s = Wqkv[2 * C:] * 64.0
    # interleave Q/K 128-col chunks: [Q0,K0,Q1,K1,...,Q5,K5,V]
    qk = np.concatenate(
        [np.stack([Wq_s.reshape(CB, 128, C)[j], Wk_s.reshape(CB, 128, C)[j]])
         for j in range(CB)], axis=0).reshape(2 * C, C)
    W8 = np.concatenate([qk, Wv_s], axis=0)                  # [3C, C]
    # wqkv8[p, pb, i, j] = W8[j, 256*pb + 128*i + p]
    wqkv8 = np.ascontiguousarray(
        W8.T.reshape(PB, 2, 128, 3 * C).transpose(2, 0, 1, 3)).astype(f8np)
    wqa = np.ascontiguousarray(wqkv8[:, :, :, 0:256])
    # wp8[p, pb, i, o] = 64*Wproj[o, f], f = 64*(4*pb+2*i+(p>=64)) + p%64
    p_idx = np.arange(128)
    fb_idx = np.arange(PB)
    i_idx = np.arange(2)
    fmap = (64 * (4 * fb_idx[None, :, None] + 2 * i_idx[None, None, :]
                  + (p_idx[:, None, None] // 64)) + (p_idx[:, None, None] % 64))
    wp8a = (64.0 * Wproj).T[fmap].astype(f8np)               # [128, PB, 2, C]
    W2 = Wproj @ Wqkv[2 * C:3 * C]
    w2T = np.ascontiguousarray(
        W2.T.reshape(CB, 128, C).transpose(1, 0, 2)).astype(np.float16)
    biasb = np.ascontiguousarray(
        np.broadcast_to(bproj[None, :], (128, C))).astype(np.float32)

    KB = NK // 128
    in_maps = []
    for b in range(B):
        xk = np.zeros((NK, C), np.float32)
        xk[:nk[b]] = x[b][kept[b]]
        x8pa = np.ascontiguousarray(
            xk.T.reshape(PB, 2, 128, NK).transpose(2, 0, 1, 3)).astype(f8np)
        xdTa = np.zeros((C, ND), np.float32)
        xdTa[:, :nd[b]] = x[b][drop[b]].T
        xdTa = np.ascontiguousarray(
            xdTa.reshape(CB, 128, ND).transpose(1, 0, 2)).astype(np.float16)
        pol8a = np.zeros((NK, H), np.float32)
        pol8a[:nk[b], :] = 1.0
        pol8a = np.ascontiguousarray(
            pol8a.reshape(KB, 128, H).transpose(1, 0, 2)).astype(f8np)
        in_maps.append({
            "x8p": x8pa, "wqkv8": wqkv8, "wqa": wqa, "wp8": wp8a,
            "xdT": xdTa, "w2T": w2T, "pol8": pol8a, "biasb": biasb,
        })

    res = run_bass_kernel_spmd(nc, in_maps, core_ids=list(range(B)),
                               trace=_trace, tmpdir=_tmpdir)

    out = np.empty((B, N, C), np.float32)
    for b in range(B):
        out[b, kept[b]] = res.results[b]["outk"][:nk[b]].astype(np.float32)
        out[b, drop[b]] = res.results[b]["outd"][:nd[b]].astype(np.float32)
    if _trace:
        kernel._last = res
    return out


# revision 78
# speedup vs baseline: 1.0149x; 1.0149x over previous
"""Policy-masked sparse attention on 8 trn2 NeuronCores — fp8 DoubleRow.

Data-parallel over B (one batch element per core).  The 768-deep matmuls
(QKV, V, T-accumulate, proj) run as fp8e4m3 DoubleRow.  The 64-deep S
matmul stays fp16 and is row-tiled: heads 2f/2f+1 occupy partitions
0:64 / 64:128 of the K/Q chunk tiles, so their score matmuls issue to row
groups (0,0)/(64,0) and run concurrently in the PE array.

Softmax post-processing is the machine bottleneck (every S element must
pass through Scalar or Vector once — the only engines with a PSUM port):
  - exp blocks split per EXP_TABLE: Scalar true Exp (fp8 out) vs Vector
    Schraudolph (E = bits((log2e*S + (7-c))*8) as int8 ≡ fp8e4m3), paired
    so the two heads of a score block mostly run on different engines.
  - V_aug carries pol in col 0, V in cols 64:128, so the denominator row
    lands on T-psum partition 0 where reciprocal_approx_fast can read it
    directly (partition-0-only op); GpSimd broadcasts it; Vector scales.
  - 4 rotating 2-bank PSUM buffers shared by score pairs, T-accumulates
    and all projection work; per-pair schedule interleaves QKV/V/drop/T
    fillers (each <=1us of PE) between score blocks.
  - ~4.8us of dummy matmuls during the input-DMA wait open the HAM clock
    gate before real work; a duplicated contiguous Q0K0 weight block plus
    per-pb x transfers shorten the critical DMA path.

Scales (power-of-2, folded on host):
  Wq' = Wq*hd^-0.5*64, Wk' = Wk*64, Wv' = Wv*64, x fp8 plain
  QKV psum = 64*q ; Q/K cast to fp16 -> 64*q ; S psum = 4096*S
  exp scale 1/4096 ; V stored fp8 64*v ; T psum = 64*num ; D = pol row
  OAT = T/D = 64*attn (fp8) ; Wp' = 64*Wproj ; proj psum = 4096*out
  final scalar_tensor_tensor: out = psum/4096 + bias
Drop path (out = x@W2.T + b) stays fp16 (fp8 measured 3.2% rel err).
Measured: 102.9us HW exec, rel err 2.8e-3 (baseline 145.5us).
"""

import math
import os
import numpy as np
import ml_dtypes

import concourse.bass as bass
import concourse.bacc as bacc
import concourse.mybir as mybir
from concourse import tile
from concourse.bass_utils import run_bass_kernel_spmd

C = 768
H = 12
HD = 64
CB = C // 128          # 6 feature blocks of 128
PB = C // 256          # 3 feature pair-blocks of 256
VW = 128               # V_aug per-head width: pol col 0, V cols 64:128
F8 = mybir.dt.float8e4
F16 = mybir.dt.float16
F32 = mybir.dt.float32
I8 = mybir.dt.int8
DR = mybir.MatmulPerfMode.DoubleRow
f8np = ml_dtypes.float8_e4m3

SCH_C = 0.05
SCH_A = math.log2(math.e) * 8.0
SCH_B = (7.0 - SCH_C) * 8.0
SDEN = 4096.0          # S psum carries 4096*S (Q,K held fp16 at 64x)

_cache = {}


def _groups(n, limit=512):
    out = []
    off = 0
    while off < n:
        g = min(limit, n - off)
        out.append((off, g))
        off += g
    return out


# exp engine split per (pair, kb): 2 chars for (even head, odd head),
# 'S' = scalar true-exp, 'V' = vector Schraudolph.
EXP_TABLE = {
    0: ["SV", "SV", "VS", "SV", "VS", "SV"],
    1: ["VS", "SV", "VS", "SV", "VS", "SV"],
    2: ["VS", "SV", "SS", "VS", "SS", "SV"],
    3: ["VS", "SV", "SS", "VS", "SS", "SV"],
    4: ["VS", "SV", "SS", "VS", "SS", "SV"],
    5: ["SS", "SV", "SS", "VS", "SS", "SV"],
}
# v_chunk cast engine per tb
VCAST_ENG = "VVSSSS"
# qkv dest-copy engine per chunk j (0..5 Q, 6..11 K): 'S'/'V'
QKV_COPY_ENG = "SSSSSS VSSSSS".replace(" ", "")


def _build(NK, ND, NKM):
    KB = NK // 128
    KBP = KB // 2
    DB = ND // 128
    nc = bacc.Bacc("TRN2", target_bir_lowering=False, debug=False,
                   num_devices=8)

    x8p = nc.dram_tensor("x8p", [128, PB, 2, NK], F8, kind="ExternalInput").ap()
    # wqkv8 col order: [Q0,K0,Q1,K1,...,Q5,K5, V]  (256-col QK pairs first)
    wqkv8 = nc.dram_tensor("wqkv8", [128, PB, 2, 3 * C], F8,
                           kind="ExternalInput").ap()
    # first-wave copy of the Q0K0 weight block, contiguous for one fast DMA
    wqa = nc.dram_tensor("wqa", [128, PB, 2, 256], F8,
                         kind="ExternalInput").ap()
    wp8 = nc.dram_tensor("wp8", [128, PB, 2, C], F8, kind="ExternalInput").ap()
    xdT = nc.dram_tensor("xdT", [128, CB, ND], F16, kind="ExternalInput").ap()
    w2T = nc.dram_tensor("w2T", [128, CB, C], F16, kind="ExternalInput").ap()
    pol8 = nc.dram_tensor("pol8", [128, NK // 128, H], F8,
                          kind="ExternalInput").ap()
    biasb = nc.dram_tensor("biasb", [128, C], F32, kind="ExternalInput").ap()
    outk = nc.dram_tensor("outk", [NK, C], F16, kind="ExternalOutput").ap()
    outd = nc.dram_tensor("outd", [ND, C], F16, kind="ExternalOutput").ap()

    GK = _groups(NK)
    GKM = _groups(NKM)
    GC = _groups(C)
    MUL = mybir.AluOpType.mult
    ADD = mybir.AluOpType.add

    with tile.TileContext(nc) as tc:
        with (
            tc.tile_pool(name="const", bufs=1) as cpool,
            tc.tile_pool(name="ins", bufs=1) as ipool,
            tc.tile_pool(name="acts", bufs=1) as apool,
            tc.tile_pool(name="work", bufs=4) as wpool,
            tc.tile_pool(name="outs", bufs=3) as opool,
            tc.tile_pool(name="ps", bufs=4, space="PSUM") as pspool,
        ):
            # ---- scalar act-table preload (hide the 2.7us exp set load) --
            warm = cpool.tile([1, 8], F32, name="warm", tag="warm")
            nc.gpsimd.memset(warm[:], 0.0)
            warm2 = cpool.tile([1, 8], F32, name="warm2", tag="warm2")
            nc.scalar.activation(warm2[:], warm[:],
                                 mybir.ActivationFunctionType.Exp)

            # ---- inputs (DMA emission order = priority order) ----
            wqa_t = ipool.tile([128, PB, 2, 256], F8, name="wqa", tag="wqa")
            nc.sync.dma_start(wqa_t[:], wqa)
            wq_t = [ipool.tile([128, 2, 3 * C], F8, name=f"wq{pb}",
                               tag=f"wq{pb}") for pb in range(PB)]
            xc_all = ipool.tile([128, PB, 2, NK], F8, name="xc", tag="xc")
            for pb in range(PB):
                nc.sync.dma_start(xc_all[:, pb:pb + 1], x8p[:, pb:pb + 1])

            # PE warm-up: >=3.4us of sustained dummy matmuls during the
            # DMA wait so HAM opens the clock gate before real work
            cw = cpool.tile([128, 128], F8, name="cw", tag="cw")
            nc.gpsimd.memset(cw[:], 0.0)
            wps = pspool.tile([128, 128], F32, name="wps", tag="s")
            for _ in range(45):
                nc.tensor.matmul(wps[0:16, :], lhsT=cw[:, 0:16],
                                 rhs=cw[:], start=True, stop=True)
            for pb in range(PB):
                nc.sync.dma_start(wq_t[pb][:, :, 256:2 * C],
                                  wqkv8[:, pb, :, 256:2 * C])
            pol_all = ipool.tile([128, KB, H], F8, name="pol", tag="pol")
            nc.sync.dma_start(pol_all[:], pol8)
            for pb in range(PB):
                nc.sync.dma_start(wq_t[pb][:, :, 2 * C:3 * C],
                                  wqkv8[:, pb, :, 2 * C:3 * C])
            xd_all = ipool.tile([128, CB, ND], F16, name="xd", tag="xd")
            nc.sync.dma_start(xd_all[:], xdT)
            w2_all = ipool.tile([128, CB, C], F16, name="w2", tag="w2")
            nc.sync.dma_start(w2_all[:], w2T)
            bias_t = cpool.tile([128, C], F32, name="bias", tag="bias")
            nc.sync.dma_start(bias_t[:], biasb[:])
            wp_all = ipool.tile([128, PB, 2, C], F8, name="wp", tag="wp")
            nc.sync.dma_start(wp_all[:], wp8)

            def xc_t(pb):
                return xc_all[:, pb:pb + 1].squeeze()

            def wp_t(pb):
                return wp_all[:, pb:pb + 1].squeeze()

            # ---- persistent intermediates ----
            QcT = [apool.tile([128, NKM], F16, name=f"q{j}", tag=f"q{j}")
                   for j in range(CB)]
            KcT = [apool.tile([128, NK], F16, name=f"k{j}", tag=f"k{j}")
                   for j in range(CB)]
            Vag = apool.tile([128, KBP * 2 * H * VW], F8, name="vag",
                             tag="vag")
            nc.gpsimd.memset(Vag[:], 0.0)
            OATP = [apool.tile([128, 2 * NK], F8, name=f"oat{fb}",
                               tag=f"oat{fb}") for fb in range(PB)]
            for fb in range(PB):
                # cols NKM:NK are never written by t_fin when NKM < NK
                nc.gpsimd.memset(OATP[fb][:], 0.0)
            ET = {}
            for hm in range(4):
                for kbp in range(KBP):
                    ET[(hm, kbp)] = apool.tile(
                        [128, 2 * NKM], I8, name=f"et{hm}_{kbp}",
                        tag=f"et{hm}_{kbp}")

            def et3(h, kbp):
                return ET[(h % 4, kbp)][:].rearrange("p (i n) -> p i n", i=2)

            def vag5():
                return Vag[:].rearrange("p (b i h s) -> p b i h s",
                                        b=KBP, i=2, h=H)

            def oat3(fb):
                return OATP[fb][:].rearrange("p (i n) -> p i n", i=2)

            def qkv_chunk(j):
                """Output chunk j in 0..11: j<6 -> Q (NKM cols), else K."""
                isq = j < CB
                grps = GKM if isq else GK
                w = NKM if isq else NK
                jj = j if isq else j - CB
                col0 = 256 * jj + (0 if isq else 128)
                ps = pspool.tile([128, NK], F32, name="qps", tag="s")
                for pb in range(PB):
                    if jj == 0:
                        lhsT = wqa_t[:, pb, :, (0 if isq else 128):
                                     (128 if isq else 256)]
                    else:
                        lhsT = wq_t[pb][:, :, col0:col0 + 128]
                    for (o, n) in grps:
                        nc.tensor.matmul(
                            ps[:, o:o + n],
                            lhsT=lhsT,
                            rhs=xc_t(pb)[:, :, o:o + n],
                            start=(pb == 0), stop=(pb == PB - 1),
                            perf_mode=DR)
                dest = QcT[jj] if isq else KcT[jj]
                if QKV_COPY_ENG[j] == "S":
                    nc.scalar.copy(dest[:, 0:w], ps[:, 0:w])
                else:
                    nc.vector.tensor_copy(dest[:, 0:w], ps[:, 0:w])

            def v_chunk(tb):
                """token-major V chunk for kept token block tb."""
                ps = pspool.tile([128, C], F32, name="vps", tag="s")
                for pb in range(PB):
                    for (o, n) in GC:
                        nc.tensor.matmul(
                            ps[:, o:o + n],
                            lhsT=xc_t(pb)[:, :, tb * 128:(tb + 1) * 128],
                            rhs=wq_t[pb][:, :, 2 * C + o:2 * C + o + n],
                            start=(pb == 0), stop=(pb == PB - 1),
                            perf_mode=DR)
                kbp, sl = tb // 2, tb % 2
                va = vag5()[:, kbp:kbp + 1, sl:sl + 1, :, :].squeeze()
                ps3 = ps[:].rearrange("p (h s) -> p h s", s=HD)
                # pol in col 0 (so D lands on psum partition 0), V in 64:128
                if VCAST_ENG[tb] == "S":
                    nc.scalar.copy(va[:, :, 64:64 + HD], ps3)
                else:
                    nc.vector.tensor_copy(va[:, :, 64:64 + HD], ps3)
                pol3 = pol_all[:, tb:tb + 1, :].squeeze().unsqueeze(-1)
                nc.gpsimd.tensor_copy(va[:, :, 0:1], pol3)

            def s_pair(p, kb):
                """Concurrent row-tiled score matmuls for heads 2p/2p+1,
                key block kb, followed by their exps (one per engine when
                the table splits them)."""
                fc = p
                pss = []
                for half in range(2):
                    rows = slice(half * 64, half * 64 + 64)
                    ps = pspool.tile([128, NKM], F32, name="sps", tag="s")
                    pss.append(ps)
                    for (o, n) in GKM:
                        nc.tensor.matmul(
                            ps[:, o:o + n],
                            lhsT=KcT[fc][rows, kb * 128:(kb + 1) * 128],
                            rhs=QcT[fc][rows, o:o + n],
                            start=True, stop=True,
                            tile_position=(half * 64, 0))
                for half in range(2):
                    h = 2 * p + half
                    ps = pss[half]
                    dst = et3(h, kb // 2)[:, kb % 2, 0:NKM].squeeze()
                    if EXP_TABLE[p][kb][half] == "S":
                        nc.scalar.activation(
                            dst.bitcast(F8), ps[:, 0:NKM],
                            mybir.ActivationFunctionType.Exp,
                            scale=1.0 / SDEN)
                    else:
                        nc.vector.tensor_scalar(dst, ps[:, 0:NKM],
                                                SCH_A / SDEN, SCH_B,
                                                op0=MUL, op1=ADD)

            def t_acc(h, pool=None):
                ptT = pspool.tile([VW, NKM], F32, name="ptT", tag="s")
                for kbp in range(KBP):
                    lhs = vag5()[:, kbp:kbp + 1, :, h:h + 1, :].squeeze()
                    e3 = et3(h, kbp)
                    for (o, n) in GKM:
                        nc.tensor.matmul(
                            ptT[:, o:o + n],
                            lhsT=lhs,
                            rhs=e3[:, :, o:o + n].bitcast(F8),
                            start=(kbp == 0), stop=(kbp == KBP - 1),
                            perf_mode=DR)
                return ptT

            def t_fin(h, ptT):
                # tail heads: column-split halves pipeline the
                # recip -> broadcast -> scale chain (latency, not throughput)
                fb, sl, r0 = h // 4, (h // 2) % 2, (h % 2) * 64
                dst = oat3(fb)[r0:r0 + 64, sl:sl + 1, 0:NKM].squeeze()
                r_sb = wpool.tile([1, NKM], F32, name="r_sb", tag="rsb")
                rb = wpool.tile([64, NKM], F32, name="rb", tag="rb")
                chunks = (0, NKM // 2) if h >= 10 else (0,)
                cw_ = NKM // len(chunks)
                for o in chunks:
                    nc.vector.reciprocal_approx_fast(
                        r_sb[:, o:o + cw_], ptT[0:1, o:o + cw_])
                    nc.gpsimd.partition_broadcast(
                        rb[:, o:o + cw_], r_sb[:, o:o + cw_], channels=64)
                    nc.vector.tensor_tensor(
                        dst[:, o:o + cw_], ptT[64:128, o:o + cw_],
                        rb[:, o:o + cw_], op=MUL)

            def t_head(h):
                t_fin(h, t_acc(h))

            def proj_start(tb, store):
                ps = pspool.tile([128, C], F32, name="pps", tag="s")
                store[tb] = ps
                for fb in range(PB - 1):
                    o3 = oat3(fb)
                    for (o, n) in GC:
                        nc.tensor.matmul(
                            ps[:, o:o + n],
                            lhsT=o3[:, :, tb * 128:(tb + 1) * 128],
                            rhs=wp_t(fb)[:, :, o:o + n],
                            start=(fb == 0), stop=False,
                            perf_mode=DR)

            def proj_finish(tb, store):
                ps = store.pop(tb)
                o3 = oat3(PB - 1)
                for (o, n) in GC:
                    nc.tensor.matmul(
                        ps[:, o:o + n],
                        lhsT=o3[:, :, tb * 128:(tb + 1) * 128],
                        rhs=wp_t(PB - 1)[:, :, o:o + n],
                        start=False, stop=True,
                        perf_mode=DR)
                ok = opool.tile([128, C], F16, name="ok", tag="ok")
                nc.vector.scalar_tensor_tensor(ok[:], ps[:], 1.0 / 4096.0,
                                               bias_t[:], op0=MUL, op1=ADD)
                nc.sync.dma_start(outk[tb * 128:(tb + 1) * 128, :], ok[:])

            def proj_kept(tb):
                store = {}
                proj_start(tb, store)
                proj_finish(tb, store)

            dstore = {}

            def proj_drop_part(td, part):
                """256-col slice `part` of drop-token block td (full cb sum,
                psum allocated and released within this call)."""
                o0 = part * 256
                ps = pspool.tile([128, 256], F32, name="dps", tag="s")
                for cb in range(CB):
                    nc.tensor.matmul(
                        ps[:],
                        lhsT=xd_all[:, cb, td * 128:(td + 1) * 128],
                        rhs=w2_all[:, cb, o0:o0 + 256],
                        start=(cb == 0), stop=(cb == CB - 1))
                if part == 0:
                    dstore[td] = opool.tile([128, C], F16, name="ok",
                                            tag="ok")
                ok = dstore[td]
                nc.vector.tensor_tensor(ok[:, o0:o0 + 256], ps[:],
                                        bias_t[:, o0:o0 + 256], op=ADD)
                if part == 2:
                    ok = dstore.pop(td)
                    nc.sync.dma_start(outd[td * 128:(td + 1) * 128, :], ok[:])

            # ---- schedule ----
            # fillers[p][kb] = list of callables issued after s_pair(p, kb)
            def dp(td, part):
                if td < DB:
                    return [lambda: proj_drop_part(td, part)]
                return []
            pstore = {}
            fillers = {
                0: [[lambda: qkv_chunk(CB + 1)], [lambda: qkv_chunk(1)],
                    [lambda: v_chunk(0)], [lambda: v_chunk(1)],
                    [lambda: v_chunk(2)], [lambda: v_chunk(3)]],
                1: [[lambda: v_chunk(4)], [lambda: v_chunk(5)],
                    [lambda: qkv_chunk(CB + 2)], [lambda: qkv_chunk(2)],
                    [lambda: t_head(0)], [lambda: t_head(1)]],
                2: [[lambda: qkv_chunk(CB + 3)], [lambda: qkv_chunk(3)],
                    dp(0, 0), [lambda: t_head(2)],
                    dp(0, 1), [lambda: t_head(3)]],
                3: [dp(0, 2), [lambda: qkv_chunk(CB + 4)],
                    [lambda: qkv_chunk(4)], [lambda: t_head(4)],
                    dp(1, 0), [lambda: t_head(5)]],
                4: [dp(1, 1), [lambda: qkv_chunk(CB + 5)],
                    dp(1, 2), [lambda: t_head(6)],
                    [lambda: qkv_chunk(5)], [lambda: t_head(7)]],
                5: [dp(2, 0), dp(2, 1),
                    [lambda: t_head(8)], dp(2, 2),
                    [lambda: t_head(9)], []],
            }
            qkv_chunk(CB + 0)      # K heads 0,1
            qkv_chunk(0)           # Q heads 0,1
            for p in range(6):
                fl = fillers[p]
                for kb in range(KB):
                    s_pair(p, kb)
                    if kb < len(fl):
                        for f in fl[kb]:
                            f()
            # tail: both T-accumulates back-to-back on PE, then the proj
            # fb0/fb1 partials run under the t_fin vector chains
            pt10 = t_acc(10)
            pt11 = t_acc(11)
            proj_start(0, pstore)
            proj_start(1, pstore)
            proj_start(2, pstore)
            t_fin(10, pt10)
            t_fin(11, pt11)
            proj_finish(0, pstore)
            proj_finish(1, pstore)
            proj_finish(2, pstore)
            for tb in range(3, KB):
                proj_kept(tb)

    nc.compile()
    return nc


def kernel(x, policy, Wqkv, Wproj, bproj, _trace=False, _tmpdir=None):
    x = np.asarray(x)
    policy = np.asarray(policy)
    Wqkv = np.asarray(Wqkv, dtype=np.float32)
    Wproj = np.asarray(Wproj, dtype=np.float32)
    bproj = np.asarray(bproj, dtype=np.float32)
    B, N, _ = x.shape
    assert B == 8 and x.shape[2] == C

    pol = policy[:, :, 0] > 0.5
    kept = [np.nonzero(pol[b])[0] for b in range(B)]
    drop = [np.nonzero(~pol[b])[0] for b in range(B)]
    nk = [len(i) for i in kept]
    nd = [len(i) for i in drop]
    NK = max(256, int(math.ceil(max(nk) / 256.0)) * 256)
    ND = max(128, int(math.ceil(max(nd) / 128.0)) * 128)
    NKM = min(NK, max(128, int(math.ceil(max(nk) / 16.0)) * 16))

    key = (NK, ND, NKM)
    if key not in _cache:
        _cache[key] = _build(NK, ND, NKM)
    nc = _cache[key]

    # ---- shared weight prep ----
    Wq_s = Wqkv[:C] * (HD ** -0.5) * 64.0
    Wk_s = Wqkv[C:2 * C] * 64.0
    Wv_s = Wqkv[2 * C:] * 64.0
    # interleave Q/K 128-col chunks: [Q0,K0,Q1,K1,...,Q5,K5,V]
    qk = np.concatenate(
        [np.stack([Wq_s.reshape(CB, 128, C)[j], Wk_s.reshape(CB, 128, C)[j]])
         for j in range(CB)], axis=0).reshape(2 * C, C)
    W8 = np.concatenate([qk, Wv_s], axis=0)                  # [3C, C]
    # wqkv8[p, pb, i, j] = W8[j, 256*pb + 128*i + p]
    wqkv8 = np.ascontiguousarray(
        W8.T.reshape(PB, 2, 128, 3 * C).transpose(2, 0, 1, 3)).astype(f8np)
    wqa = np.ascontiguousarray(wqkv8[:, :, :, 0:256])
    # wp8[p, pb, i, o] = 64*Wproj[o, f], f = 64*(4*pb+2*i+(p>=64)) + p%64
    p_idx = np.arange(128)
    fb_idx = np.arange(PB)
    i_idx = np.arange(2)
    fmap = (64 * (4 * fb_idx[None, :, None] + 2 * i_idx[None, None, :]
                  + (p_idx[:, None, None] // 64)) + (p_idx[:, None, None] % 64))
    wp8a = (64.0 * Wproj).T[fmap].astype(f8np)               # [128, PB, 2, C]
    W2 = Wproj @ Wqkv[2 * C:3 * C]
    w2T = np.ascontiguousarray(
        W2.T.reshape(CB, 128, C).transpose(1, 0, 2)).astype(np.float16)
    biasb = np.ascontiguousarray(
        np.broadcast_to(bproj[None, :], (128, C))).astype(np.float32)

    KB = NK // 128
    in_maps = []
    for b in range(B):
        xk = np.zeros((NK, C), np.float32)
        xk[:nk[b]] = x[b][kept[b]]
        x8pa = np.ascontiguousarray(
            xk.T.reshape(PB, 2, 128, NK).transpose(2, 0, 1, 3)).astype(f8np)
        xdTa = np.zeros((C, ND), np.float32)
        xdTa[:, :nd[b]] = x[b][drop[b]].T
        xdTa = np.ascontiguousarray(
            xdTa.reshape(CB, 128, ND).transpose(1, 0, 2)).astype(np.float16)
        pol8a = np.zeros((NK, H), np.float32)
        pol8a[:nk[b], :] = 1.0
        pol8a = np.ascontiguousarray(
            pol8a.reshape(KB, 128, H).transpose(1, 0, 2)).astype(f8np)
        in_maps.append({
            "x8p": x8pa, "wqkv8": wqkv8, "wqa": wqa, "wp8": wp8a,
            "xdT": xdTa, "w2T": w2T, "pol8": pol8a, "biasb": biasb,
        })

    res = run_bass_kernel_spmd(nc, in_maps, core_ids=list(range(B)),
                               trace=_trace, tmpdir=_tmpdir)

    out = np.empty((B, N, C), np.float32)
    for b in range(B):
        out[b, kept[b]] = res.results[b]["outk"][:nk[b]].astype(np.float32)
        out[b, drop[b]] = res.results[b]["outd"][:nd[b]].astype(np.float32)
    if _trace:
        kernel._last = res
    return out
